# revision 2
# baseline (speedup 1.0000x reference)
"""Trainium2 Bass kernel: autoregressive LSTM decoder (nn_DecoderAR), fp8 edition.

Reference computation (per step t over HORIZON=24):
    inp   = concat([x_t, y_prev])                      (B, 8)
    gates = inp @ W_ih.T + b_ih + h @ W_hh.T + b_hh    (B, 2048)
    i, f, g, o = split(gates); sigmoid/tanh
    c = f*c + i*g ; h = o*tanh(c)
    logit = h @ fc_w.T + fc_b ; y_prev = sigmoid(logit)
Output: logits (B, 24, 1).

Sharding: data-parallel over batch (8192 -> 8 cores x 1024), weights
replicated.  On-chip layout is transposed (batch on the matmul free dim,
gate/hid dims on partitions) so the recurrence needs no transposes.

v2 changes over the bf16 baseline:
  - W_hh (x256) and h quantized fp8e4; the dominant recurrent matmuls run
    as DoubleRow pairs (2 K=128 chunks per instruction, ~2x column rate).
    The x256 weight scale (fp8e4 normal range) is undone by the ScalarE
    activation's free scale=1/256; validated rel_err ~0.0185 in numpy sim.
  - fc head stays bf16 (fp8 there doubles the error: it feeds the output
    directly and recirculates through y), reading a bf16 copy of h.
  - b_ih+b_hh folded into the extras matmul as a x256-scaled ones-row, so
    gate activations need no per-partition bias and (f,o) sigmoids merge
    into one 2-bank ScalarE op. PSUM pairs: (i,g) and (f,o).
  - extras (K=9: y, 7 covariates, ones) packed 4x via 32-row PE strips
    (tile_position), y replicated to strips 32/64/96 by SBUF-SBUF DMA.
  - gates/c/elementwise in bf16 (2x DVE); c tanh merged over all 4 hid
    chunks; h8 produced by one merged bf16->fp8 copy.
"""

import numpy as np
import ml_dtypes

import concourse.bass as bass
import concourse.mybir as mybir
import concourse.tile as tile
from concourse import bacc
from concourse.bass_utils import run_bass_kernel_spmd

B, HORIZON, NCOV, HID = 8192, 24, 7, 512
NCORES = 8
BL = B // NCORES          # batch rows per core (1024)
P = 128
KC = HID // P             # hid chunks (4)
NMC = 4 * HID // P        # gate chunks (16)
KE = NCOV + 2             # extras rows: y + 7 covariates + ones(bias)

F32 = mybir.dt.float32
BF16 = mybir.dt.bfloat16
FP8 = mybir.dt.float8e4
AF = mybir.ActivationFunctionType
DR = mybir.MatmulPerfMode.DoubleRow
BF16NP = ml_dtypes.bfloat16
FP8NP = ml_dtypes.float8_e4m3fn

WSCALE = 256.0            # fp8 weight prescale (power of 2; exact to undo)

# gate order in the PyTorch LSTMCell layout: i, f, g, o -> mc = gate*KC + j
# PSUM pairing: (i,g) unlocks t1=i*g early; (f,o) merge into one sigmoid.
PACK_EXTRAS = True


def build_program(horizon=HORIZON, bl=BL, repeats=1, pack=None):
    """Build the single-core Bass program (identical on all cores).

    repeats>1 re-runs the whole recurrence (benchmarking only: the extra
    passes reuse the same x slots / output rows, so results are those of
    the last pass, which no longer matches the reference)."""
    bh = bl // 2  # batch half = matmul free dim (512)
    nc = bacc.Bacc(None)

    xrep = nc.declare_dram_parameter("xrep", [P, horizon, bl], BF16, False)
    h80 = nc.declare_dram_parameter("h80", [P, KC, bl], FP8, False)
    h160 = nc.declare_dram_parameter("h160", [P, KC, bl], BF16, False)
    c0 = nc.declare_dram_parameter("c0", [P, KC, bl], BF16, False)
    whh = nc.declare_dram_parameter("whh", [P, KC, 4 * HID], FP8, False)
    we = nc.declare_dram_parameter("we", [P, KC, P], BF16, False)
    fcw = nc.declare_dram_parameter("fcw", [P, KC], BF16, False)
    fcb = nc.declare_dram_parameter("fcb", [1, 1], BF16, False)
    out = nc.declare_dram_parameter("out", [horizon, bl], F32, True)

    inv = 1.0 / WSCALE

    with tile.TileContext(nc) as tc:
        with (
            tc.tile_pool(name="singles", bufs=1) as singles,
            tc.tile_pool(name="gacts", bufs=8) as gacts,
            tc.tile_pool(name="tnhs", bufs=3) as tnhs,
            tc.tile_pool(name="tails", bufs=6) as tails,
            tc.tile_pool(name="ps_g", bufs=3, space="PSUM") as ps_g,
            tc.tile_pool(name="ps_fc", bufs=2, space="PSUM") as ps_fc,
        ):
            # --- resident tensors, loaded once ---
            xs_sb = []
            for hf in range(2):
                xt = singles.tile([P, horizon, bh], BF16, tag=f"xrep{hf}", name=f"xrep{hf}")
                nc.sync.dma_start(xt[:], xrep[:, :, hf * bh:(hf + 1) * bh])
                xs_sb.append(xt)
            whh_sb = singles.tile([P, KC, 4 * HID], FP8, tag="whh", name="whh")
            nc.sync.dma_start(whh_sb[:], whh[:])
            we_sb = singles.tile([P, KC, P], BF16, tag="we", name="we")
            nc.sync.dma_start(we_sb[:], we[:])
            fcw_sb = singles.tile([P, KC], BF16, tag="fcw", name="fcw")
            nc.sync.dma_start(fcw_sb[:], fcw[:])
            fcb_sb = singles.tile([1, 1], BF16, tag="fcb", name="fcb")
            nc.sync.dma_start(fcb_sb[:], fcb[:])
            ones_sb = singles.tile([1, bh], BF16, tag="ones", name="ones")
            nc.vector.memset(ones_sb[:], 1.0)

            # ping-pong h8 (fp8 = DR rhs), h16 (bf16 = fc rhs), c (bf16)
            h8_sb = [[singles.tile([P, KC, bh], FP8, tag=f"h8{hf}_{i}", name=f"h8{hf}_{i}")
                      for i in range(2)] for hf in range(2)]
            h16_sb = [[singles.tile([P, KC, bh], BF16, tag=f"h16{hf}_{i}", name=f"h16{hf}_{i}")
                       for i in range(2)] for hf in range(2)]
            c_sb = [[singles.tile([P, KC, bh], BF16, tag=f"c{hf}_{i}", name=f"c{hf}_{i}")
                     for i in range(2)] for hf in range(2)]
            for hf in range(2):
                csl = slice(hf * bh, (hf + 1) * bh)
                nc.sync.dma_start(h8_sb[hf][0][:], h80[:, :, csl])
                nc.sync.dma_start(h16_sb[hf][0][:], h160[:, :, csl])
                nc.sync.dma_start(c_sb[hf][0][:], c0[:, :, csl])

            def emit_chains(hf, t, tg):
                """Gate chains + c update for one (half, step); returns the
                per-chunk (f,o) activation tiles needed by the h tail."""
                cur, nxt = tg % 2, (tg + 1) % 2
                xs = xs_sb[hf]
                h8cur = h8_sb[hf][cur]
                ccur, cnxt = c_sb[hf][cur], c_sb[hf][nxt]
                gfos = []
                for j in range(KC):
                    ps_ig = ps_g.tile([P, 2, bh], F32, tag="gps", name="gps")
                    ps_fo = ps_g.tile([P, 2, bh], F32, tag="gps", name="gps")
                    # slots: ps_ig = (i, g), ps_fo = (f, o)
                    plan = [(0, ps_ig, 0), (2, ps_ig, 1), (1, ps_fo, 0), (3, ps_fo, 1)]
                    for gi, ps, slot in plan:
                        mc = gi * KC + j
                        for p in range(2):
                            nc.tensor.matmul(
                                ps[:, slot, :],
                                whh_sb[:, 2 * p:2 * p + 2, mc * P:(mc + 1) * P],
                                h8cur[:, 2 * p:2 * p + 2, :],
                                start=(p == 0), stop=False,
                                perf_mode=DR,
                            )
                    # extras: 4 concurrent K=9 matmuls, one per 32-row strip
                    # (strip g serves gate g); y slot row 0, ones row 8.
                    for gi, ps, slot in plan:
                        nc.tensor.matmul(
                            ps[:, slot, :],
                            we_sb[32 * gi:32 * gi + KE, j, :],
                            xs[32 * gi:32 * gi + KE, t, :],
                            start=False, stop=True,
                            tile_position=(32 * gi, 0),
                        )
                    gig = gacts.tile([P, 2, bh], BF16, tag="gact", name="gact")
                    gfo = gacts.tile([P, 2, bh], BF16, tag="gact", name="gact")
                    nc.scalar.activation(gig[:, 0, :], ps_ig[:, 0, :], AF.Sigmoid, scale=inv)
                    nc.scalar.activation(gig[:, 1, :], ps_ig[:, 1, :], AF.Tanh, scale=inv)
                    nc.scalar.activation(gfo[:, :, :], ps_fo[:, :, :], AF.Sigmoid, scale=inv)
                    t1 = tails.tile([P, bh], BF16, tag="t1", name="t1")
                    t2 = tails.tile([P, bh], BF16, tag="t2", name="t2")
                    nc.vector.tensor_mul(t1[:], gig[:, 0, :], gig[:, 1, :])
                    nc.vector.tensor_mul(t2[:], gfo[:, 0, :], ccur[:, j, :])
                    nc.vector.tensor_add(cnxt[:, j, :], t1[:], t2[:])
                    gfos.append(gfo)
                return gfos

            def emit_h(hf, t, tg, gfos):
                """tanh(c) merged over all chunks; h16 muls; merged h8 copy."""
                nxt = (tg + 1) % 2
                cnxt = c_sb[hf][nxt]
                h16n, h8n = h16_sb[hf][nxt], h8_sb[hf][nxt]
                tnh = tnhs.tile([P, KC, bh], BF16, tag="tnh", name="tnh")
                nc.scalar.activation(tnh[:], cnxt[:], AF.Tanh)
                for j in range(KC):
                    nc.vector.tensor_mul(h16n[:, j, :], gfos[j][:, 1, :], tnh[:, j, :])
                nc.vector.tensor_copy(h8n[:], h16n[:])

            def emit_tail(hf, t, tg):
                """fc logit (+fc_b) + y recirculation for one (half, step)."""
                nxt = (tg + 1) % 2
                h16n = h16_sb[hf][nxt]
                fc_ps = ps_fc.tile([1, bh], F32, tag="fc", name="fc_ps")
                for j in range(KC):
                    nc.tensor.matmul(
                        fc_ps[:], fcw_sb[:, j:j + 1], h16n[:, j, :],
                        start=(j == 0), stop=False,
                    )
                # fc_b folded in as a rank-1 matmul against a ones row
                nc.tensor.matmul(
                    fc_ps[:], fcb_sb[:], ones_sb[:],
                    start=False, stop=True,
                )
                if tg + 1 < horizon * repeats:
                    ts = (t + 1) % horizon
                    # y slot is row 0 of strip 0; DMA-replicate to strips 1-3
                    nc.scalar.activation(
                        xs_sb[hf][0:1, ts, :], fc_ps[:], AF.Sigmoid,
                    )
                    for g in range(1, 4):
                        nc.sync.dma_start(
                            xs_sb[hf][32 * g:32 * g + 1, ts, :],
                            xs_sb[hf][0:1, ts, :],
                        )
                # bounce logit through SBUF (DMA cannot read PSUM)
                osl = tails.tile([1, bh], F32, tag="osl", name="osl")
                nc.vector.tensor_copy(osl[:], fc_ps[:])
                nc.sync.dma_start(out[t:t + 1, hf * bh:(hf + 1) * bh], osl[:])

            pending = None
            for rep in range(repeats):
                for t in range(horizon):
                    tg = rep * horizon + t
                    for hf in range(2):
                        gfos = emit_chains(hf, t, tg)
                        emit_h(hf, t, tg, gfos)
                        if pending is not None:
                            emit_tail(*pending)
                        pending = (hf, t, tg)
            emit_tail(*pending)

    nc.finalize()
    return nc


def prepare_inputs(future_x, h_enc, c_enc, y0, W_ih, W_hh, b_ih, b_hh,
                   fc_w, fc_b, horizon=HORIZON, bl=BL, ncores=NCORES,
                   pack=None):
    """Host-side shard + layout prep. Returns list of per-core input dicts."""
    future_x = np.asarray(future_x, np.float32)
    h_enc = np.asarray(h_enc, np.float32)
    c_enc = np.asarray(c_enc, np.float32)
    y0 = np.asarray(y0, np.float32)
    W_ih = np.asarray(W_ih, np.float32)
    W_hh = np.asarray(W_hh, np.float32)
    bias = (np.asarray(b_ih, np.float32) + np.asarray(b_hh, np.float32))
    fc_w = np.asarray(fc_w, np.float32)
    fc_b = np.asarray(fc_b, np.float32)

    # whh[p, k, m] = W_hh[m, k*128+p] * 256, fp8e4
    whh_host = np.ascontiguousarray(
        (W_hh.T * WSCALE).reshape(KC, P, 4 * HID).transpose(1, 0, 2)).astype(FP8NP)
    # extras weights per 32-row strip g (serving gate g), chunk j, col c:
    # row 0 = y weights, 1-7 = covariates, 8 = bias; all x256 (bf16)
    we_host = np.zeros((P, KC, P), BF16NP)
    wi = (W_ih * WSCALE).reshape(4, KC, P, NCOV + 1)  # [gate, j, c, in]
    bi = (bias * WSCALE).reshape(4, KC, P)
    for g in range(4):
        we_host[32 * g + 0] = wi[g, :, :, NCOV]           # y column
        for r in range(NCOV):
            we_host[32 * g + 1 + r] = wi[g, :, :, r]      # covariates
        we_host[32 * g + 8] = bi[g]                       # bias row
    fcw_host = np.ascontiguousarray(fc_w.reshape(KC, P).T).astype(BF16NP)
    fcb_host = np.full((1, 1), float(fc_b[0]), BF16NP)

    in_maps = []
    for core in range(ncores):
        sl = slice(core * bl, (core + 1) * bl)
        xrep_host = np.zeros((P, horizon, bl), BF16NP)
        xt = future_x[sl, :horizon].transpose(2, 1, 0).astype(BF16NP)
        y0t = y0[sl, 0].astype(BF16NP)
        for g in range(4):
            xrep_host[32 * g + 1:32 * g + 1 + NCOV] = xt  # covariate rows
            xrep_host[32 * g + 8] = 1.0                   # ones (bias) row
            xrep_host[32 * g, 0, :] = y0t                 # y slot, step 0
        ht = h_enc[sl].T.reshape(KC, P, bl).transpose(1, 0, 2)
        h8_host = np.ascontiguousarray(ht).astype(FP8NP)
        h16_host = np.ascontiguousarray(ht).astype(BF16NP)
        c0_host = np.ascontiguousarray(
            c_enc[sl].T.reshape(KC, P, bl).transpose(1, 0, 2)).astype(BF16NP)
        in_maps.append({
            "xrep": xrep_host,
            "h80": h8_host,
            "h160": h16_host,
            "c0": c0_host,
            "whh": whh_host,
            "we": we_host,
            "fcw": fcw_host,
            "fcb": fcb_host,
        })
    return in_maps


def run(inputs, trace=False, **kwargs):
    """Run on 8 NeuronCores; returns (full_output, BassKernelResults)."""
    nc = build_program()
    in_maps = prepare_inputs(**inputs)
    res = run_bass_kernel_spmd(nc, in_maps, core_ids=list(range(NCORES)),
                               trace=trace, **kwargs)
    full = np.empty((B, HORIZON, 1), np.float32)
    for core in range(NCORES):
        o = np.asarray(res.results[core]["out"], np.float32)  # (HORIZON, BL)
        full[core * BL:(core + 1) * BL, :, 0] = o.T
    return full, res


def kernel(**inputs):
    out, _ = run(inputs)
    return out
